# revision 42
# baseline (speedup 1.0000x reference)
"""Trainium2 Bass kernel for nn_Encoder_76768245448827 (sparse_attention).

v11 (746us, was 1027us): data-parallel over batch (2/core); feature-major
fused [128,G,M] f32 residual (host-pre-transposed input); fp16 matmul
operands with biases folded into K=1 ones-row matmul accumulates; exact
top-32 via f16 DVE max8+match_replace; softmax without max-subtraction;
4-way row-group-packed score matmuls (one PSUM bank per head -- concurrent
row groups sharing a bank hang the device); fused [128,4,M] mask-multiply
+ exp per nt tile; 4-way col-group-packed attn@V landing directly in catT
layout; denominators via packed P=1 ones-matmul chains into a memset bank
+ reciprocal_approx_fast + f32 block-ones broadcast matmul; LN fully
on-chip (stats matmuls into the idle cat/den banks, PE [1,128]<->[128,1]
transposes to stripe var across partitions, magic+1NR rsqrt, K=1 ones
broadcast matmuls, fused apply); act-table map patched so Exp/Ln/Square/
Copy all resolve to natural_log_exp_and_others (only Tanh swaps sets);
per-batch glue chains emitted after each batch's attention so one batch's
FFN/mish overlaps the other's attention.
"""
import math

import numpy as np

import concourse.bass as bass
import concourse.mybir as mybir
import concourse.tile as tile
from concourse import bacc
from concourse import hw_specs
from concourse.bass_utils import run_bass_kernel_spmd
from concourse.masks import make_identity

# Steer the act-table pass so Exp/Ln/Square/Copy all resolve to the one set
# that genuinely contains all four (natural_log_exp_and_others); only Tanh
# stays in exp_and_others. Set names/ids are untouched, so walrus agrees.
_ORIG_GAT = hw_specs.get_activation_tables


def _patched_gat(arch):
    tabs = {k: set(v) for k, v in _ORIG_GAT(arch).items()}
    nle = tabs.get("natural_log_exp_and_others")
    if nle:
        shared = {f for f in nle
                  if f.name.lower() in ("exp", "ln", "square", "copy")}
        for name, s in tabs.items():
            if name != "natural_log_exp_and_others":
                s -= shared
    return tabs


hw_specs.get_activation_tables = _patched_gat
bacc.get_activation_tables = _patched_gat

F32 = mybir.dt.float32
F16 = mybir.dt.float16
U32 = mybir.dt.uint32
AF = mybir.ActivationFunctionType
ALU = mybir.AluOpType
AX = mybir.AxisListType

L, HEADS, TOPK, NFFN, H = 4, 8, 32, 2, 256
B, M, D = 16, 512, 32
NCORES = 8
BPC = B // NCORES
SCALE = 1.0 / math.sqrt(D)
G = H // 128   # feature groups (2)
MT = M // 128  # m tiles (4)
LN_EPS = 1e-6
EW_EPS = 1e-5
RSQRT_MAGIC = 0x5F3759DF


def build():
    nc = bacc.Bacc(name="encoder76")

    node = nc.declare_dram_parameter("node", [BPC, G, 128, M], F32, isOutput=False)
    edge = nc.declare_dram_parameter("edge", [BPC, M, M], F16, isOutput=False)
    wd, bd = {}, {}
    for i in range(L):
        for nm in ("q", "k", "v", "o", "1", "2"):
            wd[nm, i] = nc.declare_dram_parameter(f"w{nm}{i}", [H, H], F16,
                                                  isOutput=False)
        for nm in ("q", "k", "v", "o", "1", "2"):
            bd[nm, i] = nc.declare_dram_parameter(f"b{nm}{i}", [H], F16,
                                                  isOutput=False)
    lna_d = nc.declare_dram_parameter("lna", [H], F32, isOutput=False)
    lnb_d = nc.declare_dram_parameter("lnb", [H], F32, isOutput=False)
    blk_d = nc.declare_dram_parameter("blk97", [97, 128], F32, isOutput=False)
    out = nc.declare_dram_parameter("out", [BPC, M, H], F32, isOutput=True)

    from contextlib import ExitStack
    with tile.TileContext(nc) as tc, ExitStack() as ctx:
        wpool = ctx.enter_context(tc.tile_pool(name="wpool", bufs=1))
        lwpool = ctx.enter_context(tc.tile_pool(name="lwpool", bufs=2))
        xpool = ctx.enter_context(tc.tile_pool(name="xpool", bufs=2))
        ewpool = ctx.enter_context(tc.tile_pool(name="ewpool", bufs=1))
        work = ctx.enter_context(tc.tile_pool(name="work", bufs=2))
        tpool = ctx.enter_context(tc.tile_pool(name="tpool", bufs=4))
        epool = ctx.enter_context(tc.tile_pool(name="epool", bufs=6))
        mish_pool = ctx.enter_context(tc.tile_pool(name="mish", bufs=1))
        stat_pool = ctx.enter_context(tc.tile_pool(name="stat", bufs=2))
        ps_s4 = ctx.enter_context(tc.tile_pool(name="ps_s4", bufs=1, space="PSUM"))
        ps_cat = ctx.enter_context(tc.tile_pool(name="ps_cat", bufs=1, space="PSUM"))
        ps_den = ctx.enter_context(tc.tile_pool(name="ps_den", bufs=1, space="PSUM"))
        ps_proj = ctx.enter_context(tc.tile_pool(name="ps_proj", bufs=2, space="PSUM"))

        # ---- constants ----
        ident = wpool.tile([128, 128], F32, tag="ident")
        make_identity(nc, ident)
        ident16 = wpool.tile([128, 128], F16, tag="ident16")
        nc.vector.tensor_copy(ident16, ident)
        ones_col16 = wpool.tile([128, 1], F16, tag="ones_col16")
        nc.vector.memset(ones_col16, 1.0)
        ones_col32 = wpool.tile([128, 1], F32, tag="ones_col32")
        nc.vector.memset(ones_col32, 1.0)
        ones16_row = wpool.tile([1, 128], F16, tag="ones16_row")
        nc.vector.memset(ones16_row, 1.0)
        ones_row16 = wpool.tile([1, M], F16, tag="ones_row16")
        nc.vector.memset(ones_row16, 1.0)
        magic_t = wpool.tile([128, MT], U32, tag="magic")
        nc.vector.memset(magic_t, RSQRT_MAGIC)
        lnA = wpool.tile([128, G], F32, tag="lnA")
        nc.sync.dma_start(out=lnA, in_=bass.AP(tensor=lna_d, offset=0,
                                               ap=[[1, 128], [128, G]]))
        lnB = wpool.tile([128, G], F32, tag="lnB")
        nc.sync.dma_start(out=lnB, in_=bass.AP(tensor=lnb_d, offset=0,
                                               ap=[[1, 128], [128, G]]))
        blk97 = wpool.tile([97, 128], F32, tag="blk97")
        nc.sync.dma_start(out=blk97, in_=blk_d[:, :])

        def load_layer_weights(i):
            Wl, Bl = {}, {}
            for nm in ("q", "k", "v", "o", "1", "2"):
                t0 = lwpool.tile([128, H], F16, tag=f"w{nm}_0", name=f"w{nm}_0")
                t1 = lwpool.tile([128, H], F16, tag=f"w{nm}_1", name=f"w{nm}_1")
                nc.sync.dma_start(out=t0, in_=wd[nm, i][0:128, :])
                nc.sync.dma_start(out=t1, in_=wd[nm, i][128:256, :])
                Wl[nm] = (t0, t1)
            for nm in ("q", "k", "v", "o", "1", "2"):
                # bias as an f16 [1, H] row; folded into matmuls as a K=1
                # accumulate against a ones row.
                br = lwpool.tile([1, H], F16, tag=f"b{nm}_row", name=f"b{nm}_row")
                nc.sync.dma_start(
                    out=br, in_=bd[nm, i][:].rearrange("(o h) -> o h", o=1))
                Bl[nm] = br
            return Wl, Bl

        # ---- inputs arrive pre-transposed: [BPC, G, 128, M] feature-major ----
        xT = {}
        for b in range(BPC):
            xT[b] = xpool.tile([128, G, M], F32, tag=f"x_{b}", name="x0")
            nc.sync.dma_start(out=xT[b],
                              in_=node[b, :, :, :].rearrange("g p m -> p g m"))

        # ---- edges: exact top-32 -> normalize -> transpose ----
        ewnT = {}
        for b in range(BPC):
            for nt in range(MT):
                ewnT[b, nt] = ewpool.tile([128, M], F16, tag=f"ewnT_{b}_{nt}",
                                          name="ewnT")
            for mt in range(MT):
                e = work.tile([128, M], F16, tag="edge_in", bufs=3)
                nc.sync.dma_start(out=e, in_=edge[b, 128 * mt:128 * (mt + 1), :])
                scratch = work.tile([128, M], F16, tag="topk_scratch", bufs=3)
                maxes = work.tile([128, 8], F16, tag="topk_max", bufs=4)
                cur = e
                for it in range(TOPK // 8):
                    nc.vector.max(out=maxes, in_=cur)
                    nc.vector.match_replace(out=scratch, in_to_replace=maxes,
                                            in_values=cur, imm_value=0.0)
                    cur = scratch
                ew = work.tile([128, M], F16, tag="ew", bufs=3)
                nc.gpsimd.tensor_sub(ew, e, scratch)
                rs = work.tile([128, 1], F32, tag="ew_rs", bufs=4)
                nc.vector.reduce_sum(rs, ew, axis=AX.X)
                rse = work.tile([128, 1], F32, tag="ew_rse", bufs=4)
                nc.vector.tensor_scalar(rse, rs, EW_EPS, None, op0=ALU.add)
                rec = work.tile([128, 1], F32, tag="ew_rec", bufs=4)
                nc.vector.reciprocal(rec, rse)
                ewn = work.tile([128, M], F16, tag="ewn", name="ewn", bufs=3)
                nc.vector.tensor_scalar(ewn, ew, rec, SCALE, op0=ALU.mult, op1=ALU.mult)
                for nt in range(MT):
                    tp = ps_proj.tile([128, 128], F16, tag="proj", name="tps")
                    nc.tensor.transpose(tp, ewn[:, 128 * nt:128 * (nt + 1)], ident16)
                    nc.scalar.copy(ewnT[b, nt][:, 128 * mt:128 * (mt + 1)], tp)

        # ---- layernorm: replaces the residual stream (post-norm) ----
        # Fully on-chip: stats matmuls -> [1,M] rows -> PE transpose striping
        # -> magic+1NR rsqrt on [128,MT] -> transpose back -> PE broadcast.
        def layernorm(i, b, which):
            x2 = stat_pool.tile([128, G, M], F16, tag="x2")
            nc.gpsimd.tensor_mul(x2, xT[b], xT[b])
            sum_ps = ps_cat.tile([1, M], F32, tag="catps", name="stats_sum")
            nc.tensor.matmul(sum_ps, ones_col32, xT[b][:, 0, :], start=True, stop=False)
            nc.tensor.matmul(sum_ps, ones_col32, xT[b][:, 1, :], start=False, stop=True)
            sq_ps = ps_den.tile([1, M], F32, tag="denps", name="stats_sq")
            nc.tensor.matmul(sq_ps, ones_col16, x2[:, 0, :], start=True, stop=False)
            nc.tensor.matmul(sq_ps, ones_col16, x2[:, 1, :], start=False, stop=True)
            # var/negmu rows (var = (sumsq - sum^2/H) / (H-1))
            t_row = stat_pool.tile([1, M], F32, tag="t_row")
            nc.scalar.activation(t_row, sum_ps, AF.Square)
            v1_row = stat_pool.tile([1, M], F32, tag="v1_row")
            nc.vector.tensor_scalar(v1_row, sq_ps, 1.0 / (H - 1), None,
                                    op0=ALU.mult)
            var_row = stat_pool.tile([1, M], F32, tag="var_row")
            nc.vector.scalar_tensor_tensor(var_row, t_row, -1.0 / (H * (H - 1)),
                                           v1_row, op0=ALU.mult, op1=ALU.add)
            negmu16 = stat_pool.tile([1, M], F16, tag="negmu16")
            nc.vector.tensor_scalar(negmu16, sum_ps, -1.0 / H, None, op0=ALU.mult)
            # stripe var across partitions via PE transposes
            varT_ps = ps_proj.tile([128, MT], F32, tag="proj", name="varT")
            for mt in range(MT):
                nc.tensor.transpose(varT_ps[:, mt:mt + 1],
                                    var_row[0:1, 128 * mt:128 * (mt + 1)],
                                    ident[0:1, 0:1])
            varT = stat_pool.tile([128, MT], F32, tag="varT")
            nc.vector.tensor_copy(varT, varT_ps)
            # rsqrt: magic seed + 1 Newton iteration (enough for f16 targets)
            sh = stat_pool.tile([128, MT], U32, tag="ln_sh")
            nc.vector.tensor_scalar(sh, varT.bitcast(U32), 1, None,
                                    op0=ALU.logical_shift_right)
            r_u = stat_pool.tile([128, MT], U32, tag="ln_ru")
            nc.vector.tensor_sub(r_u, magic_t, sh)
            r = r_u.bitcast(F32)
            rr = stat_pool.tile([128, MT], F32, tag="ln_rr")
            nc.vector.tensor_mul(rr, r, r)
            rrv = stat_pool.tile([128, MT], F32, tag="ln_rrv")
            nc.vector.tensor_mul(rrv, rr, varT)
            f = stat_pool.tile([128, MT], F32, tag="ln_f")
            nc.vector.tensor_scalar(f, rrv, -0.5, 1.5, op0=ALU.mult, op1=ALU.add)
            rstd16 = stat_pool.tile([128, MT], F16, tag="ln_rstd16")
            nc.vector.tensor_mul(rstd16, r, f)
            # back to row form
            rsT_ps = ps_proj.tile([1, M], F16, tag="proj", name="rsT")
            for mt in range(MT):
                nc.tensor.transpose(rsT_ps[0:1, 128 * mt:128 * (mt + 1)],
                                    rstd16[:, mt:mt + 1], ident16)
            rstd_row = stat_pool.tile([1, M], F16, tag="rstd_row")
            nc.vector.tensor_copy(rstd_row, rsT_ps)
            # broadcast to [128, M] via K=1 ones matmuls
            rstd_b = ps_proj.tile([128, M], F32, tag="proj", name="rstd_b")
            nc.tensor.matmul(rstd_b, ones16_row, rstd_row, start=True, stop=True)
            nm_b = ps_proj.tile([128, M], F32, tag="proj", name="nm_b")
            nc.tensor.matmul(nm_b, ones16_row, negmu16, start=True, stop=True)
            u = stat_pool.tile([128, G, M], F32, tag="ln_u")
            nc.vector.tensor_tensor(
                u, xT[b],
                nm_b.rearrange("p (o m) -> p o m", o=1).broadcast_to([128, G, M]),
                op=ALU.add)
            w_ = stat_pool.tile([128, G, M], F32, tag="ln_w")
            nc.vector.tensor_tensor(
                w_, u,
                rstd_b.rearrange("p (o m) -> p o m", o=1).broadcast_to([128, G, M]),
                op=ALU.mult)
            xnew = xpool.tile([128, G, M], F32, tag=f"x_{b}", name="xln")
            for g in range(G):
                nc.vector.tensor_scalar(xnew[:, g, :], w_[:, g, :],
                                        lnA[:, g:g + 1], lnB[:, g:g + 1],
                                        op0=ALU.mult, op1=ALU.add)
            xT[b] = xnew
            xn16 = stat_pool.tile([128, G, M], F16, tag="ln_xn")
            nc.scalar.copy(xn16, xnew)
            return xn16

        # ---- per-(b,ot) projection + phased mish (bias folded into PE) ----
        def proj_one(w0, w1, brow, r0, r1, ot, name):
            ps = ps_proj.tile([128, M], F32, tag="proj", name=name)
            osl = bass.ts(ot, 128)
            nc.tensor.matmul(ps, w0[:, osl], r0, start=True, stop=False)
            nc.tensor.matmul(ps, w1[:, osl], r1, start=False, stop=False)
            nc.tensor.matmul(ps, brow[:, osl], ones_row16, start=False, stop=True)
            return ps

        def mish_one(ps, out_ap):
            u = mish_pool.tile([128, M], F32, tag="mish_u", name="u", bufs=4)
            nc.scalar.activation(u, ps, AF.Exp)
            z = mish_pool.tile([128, M], F16, tag="mish_z", name="z", bufs=4)
            nc.vector.tensor_copy(z, ps)
            sp = mish_pool.tile([128, M], F16, tag="mish_sp", name="sp", bufs=4)
            nc.scalar.activation(sp, u, AF.Ln, bias=1.0)
            th = mish_pool.tile([128, M], F16, tag="mish_th", name="th", bufs=4)
            nc.scalar.activation(th, sp, AF.Tanh)
            nc.gpsimd.tensor_mul(out_ap, z, th)

        # ---- layers (stage-major over b) ----
        for i in range(NL):
            W, BIAS = load_layer_weights(i)
            XN1, QT, VV = {}, {}, {}
            for b in range(BPC):
                XN1[b] = layernorm(i, b, "ln1")
            for b in range(BPC):
                xn = XN1[b]
                xn0, xn1 = xn[:, 0, :], xn[:, 1, :]
                qT, kT = [], []
                for j in range(G):
                    osl = bass.ts(j, 128)
                    qps = ps_proj.tile([128, M], F32, tag="proj", name="qkv_ps")
                    nc.tensor.matmul(qps, W["q"][0][:, osl], xn0, start=True, stop=False)
                    nc.tensor.matmul(qps, W["q"][1][:, osl], xn1, start=False, stop=False)
                    nc.tensor.matmul(qps, BIAS["q"][:, osl], ones_row16,
                                     start=False, stop=True)
                    qt = work.tile([128, M], F16, tag=f"qT{j}", name="qt")
                    nc.vector.tensor_copy(qt, qps)
                    qT.append(qt)
                    kps = ps_proj.tile([128, M], F32, tag="proj", name="qkv_ps")
                    nc.tensor.matmul(kps, W["k"][0][:, osl], xn0, start=True, stop=False)
                    nc.tensor.matmul(kps, W["k"][1][:, osl], xn1, start=False, stop=False)
                    nc.tensor.matmul(kps, BIAS["k"][:, osl], ones_row16,
                                     start=False, stop=True)
                    kt = work.tile([128, M], F16, tag=f"kT{j}", name="kt")
                    nc.vector.tensor_copy(kt, kps)
                    kT.append(kt)
                V = []
                for mt in range(MT):
                    msl = bass.ts(mt, 128)
                    vps = ps_proj.tile([128, H], F32, tag="proj", name="v_ps")
                    nc.tensor.matmul(vps, xn0[:, msl], W["v"][0], start=True, stop=False)
                    nc.tensor.matmul(vps, xn1[:, msl], W["v"][1], start=False, stop=False)
                    nc.tensor.matmul(vps, ones_row16[:, msl], BIAS["v"],
                                     start=False, stop=True)
                    vt = work.tile([128, HEADS, D], F16, tag=f"V{mt}", name="vt")
                    nc.vector.tensor_copy(
                        vt, vps.rearrange("p (h d) -> p h d", h=HEADS))
                    V.append(vt)
                QT[b] = (qT, kT)
                VV[b] = V

            for b in range(BPC):
                qT, kT = QT[b]
                V = VV[b]
                cts = []
                for quad in range(2):
                    E = []
                    for nt in range(MT):
                        sps4 = ps_s4.tile([128, 4, M], F32, tag="sps", name="sps")
                        for r in range(4):
                            rsl = bass.ds(32 * r, 32)
                            nc.tensor.matmul(
                                sps4[:, r, :],
                                kT[quad][rsl, bass.ts(nt, 128)],
                                qT[quad][rsl, :],
                                start=True, stop=True,
                                tile_position=(32 * r, 0))
                        tb = tpool.tile([128, 4, M], F16, tag="t_big")
                        nc.vector.tensor_tensor(
                            tb, sps4,
                            ewnT[b, nt].rearrange("p (o m) -> p o m", o=1)
                            .broadcast_to([128, 4, M]),
                            op=ALU.mult)
                        eb = epool.tile([128, 4, M], F16, tag="E_big")
                        nc.scalar.activation(eb, tb, AF.Exp)
                        E.append(eb)
                    # attn@V: 4-way col-packed, lands in catT layout
                    catps = ps_cat.tile([128, M], F32, tag="catps")
                    denps = ps_den.tile([128, M], F32, tag="denps")
                    nc.vector.memset(denps, 1.0)
                    for r in range(4):
                        h = 4 * quad + r
                        for nt in range(MT):
                            esl = E[nt][:, r, :]
                            nc.tensor.matmul(
                                catps[bass.ds(32 * r, 32), :],
                                V[nt][:, h, :], esl,
                                start=(nt == 0), stop=(nt == MT - 1),
                                tile_position=(0, 32 * r))
                    for r in range(4):
                        for nt in range(MT):
                            esl = E[nt][:, r, :]
                            nc.tensor.matmul(
                                denps[bass.ds(32 * r, 1), :],
                                ones_col16, esl,
                                start=(nt == 0), stop=(nt == MT - 1),
                                tile_position=(0, 32 * r))
                    rec = work.tile([97, M], F32, tag="rec", name="rec")
                    nc.vector.reciprocal_approx_fast(out=rec, in_=denps[0:97, :])
                    rb_ps = ps_proj.tile([128, M], F32, tag="proj", name="rb_ps")
                    nc.tensor.matmul(rb_ps, blk97, rec, start=True, stop=True)
                    rb16 = work.tile([128, M], F16, tag=f"rb16{quad}", name="rb16")
                    nc.vector.tensor_copy(rb16, rb_ps)
                    ct = work.tile([128, M], F16, tag=f"ct{quad}", name="ct")
                    nc.vector.tensor_mul(ct, catps, rb16)
                    cts.append(ct)

                # ---- glue for this batch (overlaps the other batch's attn) ----
                am2 = mish_pool.tile([128, G, M], F32, tag="mish_am", name="am2",
                                     bufs=2)
                for ot in range(G):
                    ops_ = proj_one(W["o"][0], W["o"][1], BIAS["o"],
                                    cts[0], cts[1], ot, "o_ps")
                    mish_one(ops_, am2[:, ot, :])
                xnew = xpool.tile([128, G, M], F32, tag=f"x_{b}", name="xres")
                nc.gpsimd.tensor_add(xnew, xT[b], am2)
                xT[b] = xnew
                xn2 = layernorm(i, b, "ln2")
                yt2 = work.tile([128, G, M], F16, tag=f"yt{b}", name="yt2")
                for ot in range(G):
                    f1 = proj_one(W["1"][0], W["1"][1], BIAS["1"],
                                  xn2[:, 0, :], xn2[:, 1, :], ot, "f_ps")
                    mish_one(f1, yt2[:, ot, :])
                ym2 = mish_pool.tile([128, G, M], F32, tag="mish_am", name="ym2",
                                     bufs=2)
                for ot in range(G):
                    f2 = proj_one(W["2"][0], W["2"][1], BIAS["2"],
                                  yt2[:, 0, :], yt2[:, 1, :], ot, "f2_ps")
                    mish_one(f2, ym2[:, ot, :])
                xnew = xpool.tile([128, G, M], F32, tag=f"x_{b}", name="xres2")
                nc.gpsimd.tensor_add(xnew, xT[b], ym2)
                xT[b] = xnew

        # ---- output ----
        for b in range(BPC):
            for mt in range(MT):
                ot_sb = work.tile([128, H], F32, tag="out_sb")
                for g in range(G):
                    tp = ps_proj.tile([128, 128], F32, tag="proj", name="tps")
                    nc.tensor.transpose(tp, xT[b][:, g, bass.ts(mt, 128)], ident)
                    nc.vector.tensor_copy(ot_sb[:, bass.ts(g, 128)], tp)
                nc.sync.dma_start(out=out[b, 128 * mt:128 * (mt + 1), :], in_=ot_sb)

    nc.finalize()
    return nc


_NC_CACHE = {}
DEBUG = False
NL = L
TRACE = False
LAST_EXEC_NS = None
LAST_RESULTS = None


def _get_nc():
    if "nc" not in _NC_CACHE:
        _NC_CACHE["nc"] = build()
    return _NC_CACHE["nc"]


def _prep_weights(attn_W, attn_b, ffn_W, ffn_b, ln_a, ln_b):
    ws = {}
    for i in range(L):
        ws[f"wq{i}"] = attn_W[i, 0].T.astype(np.float16)
        ws[f"wk{i}"] = attn_W[i, 1].T.astype(np.float16)
        ws[f"wv{i}"] = attn_W[i, 2].T.astype(np.float16)
        ws[f"wo{i}"] = attn_W[i, 3].T.astype(np.float16)
        ws[f"w1{i}"] = ffn_W[i, 0].T.astype(np.float16)
        ws[f"w2{i}"] = ffn_W[i, 1].T.astype(np.float16)
        ws[f"bq{i}"] = attn_b[i, 0].astype(np.float16)
        ws[f"bk{i}"] = attn_b[i, 1].astype(np.float16)
        ws[f"bv{i}"] = attn_b[i, 2].astype(np.float16)
        ws[f"bo{i}"] = attn_b[i, 3].astype(np.float16)
        ws[f"b1{i}"] = ffn_b[i, 0].astype(np.float16)
        ws[f"b2{i}"] = ffn_b[i, 1].astype(np.float16)
    ws["lna"] = ln_a.astype(np.float32)
    ws["lnb"] = ln_b.astype(np.float32)
    blk = np.zeros((97, 128), np.float32)
    for hh in range(4):
        blk[32 * hh, 32 * hh:32 * (hh + 1)] = 1.0
    ws["blk97"] = blk
    return ws


def kernel(node_features, edge_features, masks, attn_W, attn_b, ffn_W, ffn_b,
           ln_a, ln_b):
    node_features = np.asarray(node_features, dtype=np.float32)
    node_T = np.ascontiguousarray(
        node_features.reshape(B, M, G, 128).transpose(0, 2, 3, 1))
    edge_features = np.asarray(edge_features, dtype=np.float16)
    ws = _prep_weights(np.asarray(attn_W), np.asarray(attn_b),
                       np.asarray(ffn_W), np.asarray(ffn_b),
                       np.asarray(ln_a), np.asarray(ln_b))
    nc = _get_nc()
    in_maps = []
    for c in range(NCORES):
        m = {"node": node_T[BPC * c:BPC * (c + 1)],
             "edge": edge_features[BPC * c:BPC * (c + 1)]}
        m.update(ws)
        in_maps.append(m)
    res = run_bass_kernel_spmd(nc, in_maps, list(range(NCORES)), trace=TRACE)
    global LAST_EXEC_NS, LAST_RESULTS
    LAST_EXEC_NS = res.exec_time_ns
    LAST_RESULTS = res
    return np.concatenate([res.results[c]["out"] for c in range(NCORES)], axis=0)


if __name__ == "__main__":
    build()
    print("build OK")
